# revision 1
# baseline (speedup 1.0000x reference)
"""Causal self-attention (fused QKV + RoPE + causal softmax + out-proj) on 8
Trainium2 NeuronCores.

Sharding: tensor-parallel by heads. 16 heads / 8 cores = 2 heads per core.
Each core computes q/k/v projections for its 2 heads over the full sequence
(column-parallel c_attn), RoPE, causal attention, producing y^T for its head
slice. Two per-batch AllToAlls reshard y from head-split to row-split (the
batch-0 exchange overlaps batch-1 attention), after which each core computes
the output projection for its 2x256 rows (row-parallel c_proj without an
allreduce: the A2A moves head channels, not partial sums).

Device layout notes:
  - x is passed pre-transposed (xT [C, B*T]) so every matmul's contraction
    dim lands on the SBUF partition axis without on-device transposes.
  - S is computed transposed (S^T[k, q] = kT.T @ qT) flash-style per key
    block, so softmax normalizers come for free from an augmented V matmul
    ([V | 1]) and no P^T transpose is needed for the A*V matmul.
  - Softmax skips max-subtraction: |S|max ~ 6.5 for this problem, exp is
    safely in fp32 range.
  - Matmul operands use float32r (single-pass PE) by default; fp32 runs the
    PE in 2-pass mode at half throughput (set mm_r=False for full fp32).
  - The attention (kb, chunk) loop is software-pipelined: each iteration
    emits chunk i's S-matmuls, then chunk i-1's delayed A*V matmuls, so the
    exp (ACT) of a chunk hides under the next chunk's PE work.
"""

import sys

sys.path.insert(0, "/opt/trn_rl_repo")

import numpy as np

import concourse.bass as bass
import concourse.mybir as mybir
import concourse.tile as tile
from concourse import bacc
from concourse.bass_utils import run_bass_kernel_spmd
from concourse.masks import make_identity

B, T, C = 2, 2048, 1024
H, HD = 16, 64
HALF = HD // 2  # 32
NCORES = 8
HPC = H // NCORES  # 2 heads per core
ROWS = B * T  # 4096
DH = HPC * HD  # 128 channels per core
RPB = T // NCORES  # 256 rows per (core, batch)
ROPE_BASE = 10000.0
DT = mybir.dt.float32
FP = np.float32

KB = T // 128  # 16 key blocks per batch
QCH = 1024  # attention strip chunk width


def _build_module(use_collective=True, mm_r=True):
    # mm_r: use float32r (single-pass PE matmul, ~tf32 precision) for matmul
    # operands; plain float32 runs 2-pass at ~half throughput.
    MDT = mybir.dt.float32r if mm_r else DT
    nc = bacc.Bacc("TRN2", target_bir_lowering=False, debug=False,
                   num_devices=NCORES)

    xT_t = nc.dram_tensor("xT", [C, ROWS], MDT, kind="ExternalInput")
    wq_t = nc.dram_tensor("wq", [C, DH], MDT, kind="ExternalInput")
    wk_t = nc.dram_tensor("wk", [C, DH], MDT, kind="ExternalInput")
    wv_t = nc.dram_tensor("wv", [C, DH], MDT, kind="ExternalInput")
    bq_t = nc.dram_tensor("bq", [1, DH], MDT, kind="ExternalInput")
    bk_t = nc.dram_tensor("bk", [1, DH], MDT, kind="ExternalInput")
    bv_t = nc.dram_tensor("bv", [1, DH], MDT, kind="ExternalInput")
    wp_t = nc.dram_tensor("wp", [C, C], MDT, kind="ExternalInput")
    bp_t = nc.dram_tensor("bp", [1, C], MDT, kind="ExternalInput")
    ones_t = nc.dram_tensor("ones512", [1, 512], MDT, kind="ExternalInput")
    ropeC_t = nc.dram_tensor("ropeC", [DH, ROWS], DT, kind="ExternalInput")
    ropeS_t = nc.dram_tensor("ropeS", [DH, ROWS], DT, kind="ExternalInput")
    # rows 0:RPB = batch-0 rows [RPB*c, RPB*(c+1)), rows RPB: = batch-1 same
    out_t = nc.dram_tensor("out", [2 * RPB, C], DT, kind="ExternalOutput")

    NCI = C // 128  # 8 contraction chunks

    with tile.TileContext(nc) as tc:
        with (
            tc.tile_pool(name="persist", bufs=1) as pp,
            tc.tile_pool(name="dram", bufs=1, space="DRAM") as dp,
        ):
            ident = pp.tile([128, 64], DT, tag="ident")
            make_identity(nc, ident[0:64, :])
            nc.vector.tensor_copy(ident[64:128, :], ident[0:64, :])
            ones_row = pp.tile([1, 512], MDT, tag="ones_row")
            bq = pp.tile([1, DH], MDT, tag="bq")
            bk = pp.tile([1, DH], MDT, tag="bk")
            bv = pp.tile([1, DH], MDT, tag="bv")

            # per-batch A2A buffers: shard j of a2a_in[b] -> core j, which
            # then holds all head channels for batch-b rows [RPB*j, RPB*j+RPB)
            a2a_in = [dp.tile([NCORES, DH, RPB], MDT, tag=f"a2a_in{b}",
                              name=f"a2a_in{b}") for b in range(B)]
            a2a_out = [dp.tile([NCORES, DH, RPB], MDT, tag=f"a2a_out{b}",
                               name=f"a2a_out{b}") for b in range(B)]

            wp_sb = [pp.tile([128, C], MDT, tag=f"wp{ci}", name=f"wp{ci}")
                     for ci in range(NCI)]
            bp = pp.tile([1, C], MDT, tag="bp")
            yr = [pp.tile([128, NCORES, RPB], MDT, tag=f"yr{b}",
                          name=f"yr{b}") for b in range(B)]

            with tc.tile_pool(name="p12", bufs=1) as p12:
                qT = p12.tile([DH, ROWS], MDT, tag="qT")
                kT = p12.tile([DH, ROWS], MDT, tag="kT")
                # V_all[:, (tb*2 + h), 0:64] = V rows for global 128-token
                # block tb, head h; col 64 = ones (softmax denominator).
                V_all = p12.tile([128, 2 * KB * HPC, HD + 1], MDT, tag="V_all")
                nc.vector.memset(V_all[:, :, HD:HD + 1].bitcast(DT), 1.0)
                yT = p12.tile([DH, ROWS], MDT, tag="yT")

                # ---------------- phase 1: qkv projection + rope ----------
                with (
                    nc.named_scope("qkv"),
                    tc.tile_pool(name="ph1", bufs=1) as ph1,
                    tc.tile_pool(name="ph1ps", bufs=1, space="PSUM") as ph1ps,
                ):
                    wq_sb = [ph1.tile([128, DH], MDT, tag=f"wq{ci}", name=f"wq{ci}") for ci in range(NCI)]
                    wk_sb = [ph1.tile([128, DH], MDT, tag=f"wk{ci}", name=f"wk{ci}") for ci in range(NCI)]
                    wv_sb = [ph1.tile([128, DH], MDT, tag=f"wv{ci}", name=f"wv{ci}") for ci in range(NCI)]
                    # rope tables on the scalar HWDGE ring so they don't
                    # delay the first xq chunk on the sync ring
                    C_sb = ph1.tile([DH, ROWS], DT, tag="ropeC")
                    S_sb = ph1.tile([DH, ROWS], DT, tag="ropeS")
                    nc.scalar.dma_start(C_sb[:], ropeC_t[:])
                    nc.scalar.dma_start(S_sb[:], ropeS_t[:])

                    PART = [1, 0, 3, 2]  # rope half-rotation partner groups
                    P1C = 512  # phase-1 chunk width (1 PSUM bank)
                    for Q in range(ROWS // P1C):
                        cols = slice(Q * P1C, (Q + 1) * P1C)
                        qps = ph1ps.tile([128, P1C], DT, tag="qps", bufs=2)
                        kps = ph1ps.tile([128, P1C], DT, tag="kps", bufs=2)
                        vps = ph1ps.tile([128, P1C], DT, tag="vps", bufs=2)
                        with tc.tile_pool(name=f"xq{Q}", bufs=4) as xqp:
                            for ci in range(NCI):
                                if Q == 0:
                                    # weight loads interleaved with first use
                                    sl = slice(ci * 128, (ci + 1) * 128)
                                    nc.sync.dma_start(wq_sb[ci][:], wq_t[sl, :])
                                    nc.sync.dma_start(wk_sb[ci][:], wk_t[sl, :])
                                    nc.sync.dma_start(wv_sb[ci][:], wv_t[sl, :])
                                xq = xqp.tile([128, P1C], MDT, tag="xq")
                                nc.sync.dma_start(
                                    xq[:], xT_t[ci * 128:(ci + 1) * 128, cols])
                                if Q == 0 and ci == 0:
                                    # small constants: after the critical
                                    # first weight/xq loads
                                    nc.sync.dma_start(ones_row[:], ones_t[:])
                                    nc.sync.dma_start(bq[:], bq_t[:])
                                    nc.sync.dma_start(bk[:], bk_t[:])
                                    nc.sync.dma_start(bv[:], bv_t[:])
                                st = ci == 0
                                nc.tensor.matmul(qps[:], wq_sb[ci][:], xq[:],
                                                 start=st, stop=False)
                                nc.tensor.matmul(kps[:], wk_sb[ci][:], xq[:],
                                                 start=st, stop=False)
                                nc.tensor.matmul(vps[:], wv_sb[ci][:], xq[:],
                                                 start=st, stop=False)
                        nc.tensor.matmul(qps[:], bq[:], ones_row[:],
                                         start=False, stop=True)
                        nc.tensor.matmul(kps[:], bk[:], ones_row[:],
                                         start=False, stop=True)
                        nc.tensor.matmul(vps[:], bv[:], ones_row[:],
                                         start=False, stop=True)

                        # rope: out = q*C + rot32(q)*S, fused with psum->sbuf
                        for ps_tile, dst in ((qps, qT), (kps, kT)):
                            ta = ph1.tile([128, P1C], DT, tag="ta", bufs=2)
                            tb_ = ph1.tile([128, P1C], DT, tag="tb", bufs=2)
                            nc.vector.tensor_tensor(
                                ta[:], ps_tile[:], C_sb[:, cols],
                                mybir.AluOpType.mult)
                            for g in range(4):
                                gs = slice(32 * g, 32 * g + 32)
                                prt = slice(32 * PART[g], 32 * PART[g] + 32)
                                nc.vector.tensor_tensor(
                                    tb_[gs, :], ps_tile[prt, :], S_sb[gs, cols],
                                    mybir.AluOpType.mult)
                            nc.gpsimd.tensor_tensor(
                                dst[:, cols], ta[:], tb_[:],
                                mybir.AluOpType.add)

                        # V: copy out (ACT) then transpose into V_all
                        vt = ph1.tile([128, P1C], DT, tag="vt", bufs=2)
                        nc.scalar.copy(vt[:], vps[:])
                        for tb in range(P1C // 128):
                            gtb = Q * (P1C // 128) + tb  # global 128-token blk
                            for h in range(HPC):
                                vap = ph1ps.tile([128, HD], DT, tag="vap",
                                                 bufs=2)
                                nc.tensor.transpose(
                                    vap[:],
                                    vt[h * HD:(h + 1) * HD,
                                       tb * 128:(tb + 1) * 128],
                                    ident[h * HD:(h + 1) * HD, :])
                                nc.scalar.copy(
                                    V_all[:, gtb * HPC + h, 0:HD], vap[:])

                # load w_proj during attention (off the startup critical path)
                for ci in range(NCI):
                    nc.sync.dma_start(wp_sb[ci][:],
                                      wp_t[ci * 128:(ci + 1) * 128, :])
                nc.sync.dma_start(bp[:], bp_t[:])

                # ---------------- phase 2: causal attention ---------------
                # Software-pipelined: `delayed` holds closures (prev chunk's
                # A*V matmuls, unit finalizes, per-batch A2A staging) that are
                # drained one per chunk so ACT/DVE work hides under PE.
                with (
                    nc.named_scope("attn"),
                    tc.tile_pool(name="ph2", bufs=1) as ph2,
                    tc.tile_pool(name="ph2ps", bufs=1, space="PSUM") as ph2ps,
                ):
                    from collections import deque
                    delayed = deque()

                    def drain_one():
                        if delayed:
                            delayed.popleft()()

                    def drain_all():
                        while delayed:
                            delayed.popleft()()

                    def make_finalize(oacc_u, h_u, bT_u, qb_u):
                        hp_u = slice(h_u * HD, (h_u + 1) * HD)

                        def fin():
                            linv = ph2.tile([1, 512], MDT, tag="linv",
                                            bufs=3, name="linv")
                            with nc.allow_low_precision(
                                    reason="softmax recip in fp32r"):
                                nc.vector.reciprocal(
                                    linv[:], oacc_u[qb_u][HD:HD + 1, :])
                            rps = ph2ps.tile([64, 512], DT, tag="strip",
                                             bufs=2, name="rps")
                            nc.tensor.matmul(rps[:], ones_row[:, 0:64],
                                             linv[:], start=True, stop=True)
                            rsb = ph2.tile([128, 512], DT, tag="rsb",
                                           bufs=3, name="rsb")
                            nc.vector.tensor_copy(rsb[hp_u, :], rps[:])
                            nc.vector.tensor_tensor(
                                yT[hp_u, bT_u + qb_u * 512:
                                   bT_u + (qb_u + 1) * 512],
                                oacc_u[qb_u][0:HD, :], rsb[hp_u, :],
                                mybir.AluOpType.mult)
                        return fin

                    def make_a2a(b_u):
                        def stage():
                            for j in range(NCORES):
                                nc.sync.dma_start(
                                    a2a_in[b_u][j],
                                    yT[:, b_u * T + j * RPB:
                                       b_u * T + (j + 1) * RPB])
                            if use_collective:
                                nc.gpsimd.collective_compute(
                                    "AllToAll", mybir.AluOpType.bypass,
                                    replica_groups=[list(range(NCORES))],
                                    ins=[a2a_in[b_u].opt()],
                                    outs=[a2a_out[b_u].opt()])
                            else:
                                nc.sync.dma_start(a2a_out[b_u][:],
                                                  a2a_in[b_u][:])
                            if b_u == 0:
                                # yr[1] is loaded mid-proj: keeping it out of
                                # the sync ring here lets proj(b0) start while
                                # the batch-1 AllToAll is still in flight
                                for ci in range(NCORES):
                                    nc.sync.dma_start(yr[b_u][:, ci, :],
                                                      a2a_out[b_u][ci])
                        return stage

                    for b in range(B):
                        for h in range(HPC):
                            hp = slice(h * HD, (h + 1) * HD)
                            bT = b * T
                            oacc = [ph2ps.tile([HD + 1, 512], DT,
                                               tag=f"oacc{qb}",
                                               name=f"oacc{qb}")
                                    for qb in range(T // 512)]
                            for kb in range(KB):
                                qs = kb * 128
                                lhs_k = kT[hp, bT + qs:bT + qs + 128]
                                off = qs
                                while off < T:
                                    cw = min(QCH, T - off)
                                    qoff = off
                                    off += cw
                                    sps = ph2ps.tile([128, QCH], DT,
                                                     tag="strip", bufs=2,
                                                     name="sps")
                                    for po in range(0, cw, 512):
                                        w = min(512, cw - po)
                                        nc.tensor.matmul(
                                            sps[:, po:po + w], lhs_k,
                                            qT[hp, bT + qoff + po:
                                               bT + qoff + po + w],
                                            start=True, stop=True)
                                    # bounded 2-deep pipeline: spread fin
                                    # bursts across a couple of chunks
                                    while len(delayed) > 2:
                                        delayed.popleft()()
                                    psb = ph2.tile([128, QCH], MDT, tag="psb",
                                                   bufs=4, name="psb")
                                    nc.scalar.activation(
                                        psb[:, 0:cw], sps[:, 0:cw],
                                        mybir.ActivationFunctionType.Exp,
                                        scale=1.0 / float(np.sqrt(HD)))
                                    if qoff == qs:
                                        # zero strict upper triangle (k > q)
                                        nc.gpsimd.affine_select(
                                            out=psb[:, 0:128],
                                            in_=psb[:, 0:128],
                                            compare_op=mybir.AluOpType.is_ge,
                                            fill=0.0, base=0,
                                            pattern=[[1, 128]],
                                            channel_multiplier=-1)

                                    def make_av(oacc_u=oacc, psb_u=psb,
                                                kb_u=kb, qoff_u=qoff,
                                                cw_u=cw, b_u=b, h_u=h):
                                        def av():
                                            vidx = ((b_u * KB + kb_u) * HPC
                                                    + h_u)
                                            for qb in range(T // 512):
                                                lo = max(qoff_u, qb * 512)
                                                hi = min(qoff_u + cw_u,
                                                         qb * 512 + 512)
                                                if lo >= hi:
                                                    continue
                                                nc.tensor.matmul(
                                                    oacc_u[qb][:,
                                                               lo - qb * 512:
                                                               hi - qb * 512],
                                                    V_all[:, vidx, :],
                                                    psb_u[:, lo - qoff_u:
                                                          hi - qoff_u],
                                                    start=(kb_u == 0),
                                                    stop=(kb_u == 4 * qb + 3))
                                        return av

                                    delayed.append(make_av())
                            for qb in range(T // 512):
                                delayed.append(make_finalize(oacc, h, bT, qb))
                        delayed.append(make_a2a(b))
                    drain_all()

            # ---------------- phase 3: output projection ------------------
            with (
                nc.named_scope("proj"),
                tc.tile_pool(name="ph3", bufs=1) as ph3,
                tc.tile_pool(name="ph3ps", bufs=2, space="PSUM") as ph3ps,
            ):
                for b in range(B):
                    if b == 1:
                        for ci in range(NCORES):
                            nc.sync.dma_start(yr[1][:, ci, :], a2a_out[1][ci])
                    for tb in range(RPB // 128):
                        for co in range(C // 512):
                            pps = ph3ps.tile([128, 512], DT, tag="pps",
                                             name="pps")
                            for ci in range(NCI):
                                nc.tensor.matmul(
                                    pps[:],
                                    yr[b][:, ci, tb * 128:(tb + 1) * 128],
                                    wp_sb[ci][:, co * 512:(co + 1) * 512],
                                    start=(ci == 0), stop=False)
                            nc.tensor.matmul(pps[:], ones_row[:, 0:128],
                                             bp[:, co * 512:(co + 1) * 512],
                                             start=False, stop=True)
                            osb = ph3.tile([128, 512], DT, tag="osb", bufs=2,
                                           name="osb")
                            nc.vector.tensor_copy(osb[:], pps[:])
                            nc.sync.dma_start(
                                out_t[b * RPB + tb * 128:
                                      b * RPB + (tb + 1) * 128,
                                      co * 512:(co + 1) * 512], osb[:])

    nc.compile()
    return nc


_NC_CACHE = None


def _get_module():
    global _NC_CACHE
    if _NC_CACHE is None:
        _NC_CACHE = _build_module()
    return _NC_CACHE


def _rope_tables():
    inv = ROPE_BASE ** (-np.arange(HALF, dtype=np.float64) / HALF)
    tt = np.arange(T, dtype=np.float64)
    ang = tt[None, :] * inv[:, None]  # [32, T]
    cos = np.cos(ang).astype(FP)  # [32, T]
    sin = np.sin(ang).astype(FP)
    Cq = np.concatenate([cos, cos], axis=0)  # [64, T] (p%32 freq)
    Sq = np.concatenate([-sin, sin], axis=0)
    # duplicate for the HPC heads (partition dim) and tile across B along
    # columns (t_global = b*T + tt)
    Cq = np.tile(Cq, (HPC, B))
    Sq = np.tile(Sq, (HPC, B))
    return np.ascontiguousarray(Cq), np.ascontiguousarray(Sq)


def kernel(x, w_attn, b_attn, w_proj, b_proj, _trace=False):
    x = np.asarray(x, dtype=FP)
    w_attn = np.asarray(w_attn, dtype=FP)
    b_attn = np.asarray(b_attn, dtype=FP)
    w_proj = np.asarray(w_proj, dtype=FP)
    b_proj = np.asarray(b_proj, dtype=FP)

    xT = np.ascontiguousarray(x.reshape(ROWS, C).T)  # [C, ROWS]
    ropeC, ropeS = _rope_tables()
    bp = np.ascontiguousarray(b_proj[None, :])
    ones512 = np.ones((1, 512), FP)

    in_maps = []
    for c in range(NCORES):
        h0 = HPC * c
        cols = slice(h0 * HD, (h0 + HPC) * HD)  # this core's head channels
        in_maps.append({
            "xT": xT,
            "wq": np.ascontiguousarray(w_attn[:, 0 * C:1 * C][:, cols]),
            "wk": np.ascontiguousarray(w_attn[:, 1 * C:2 * C][:, cols]),
            "wv": np.ascontiguousarray(w_attn[:, 2 * C:3 * C][:, cols]),
            "bq": np.ascontiguousarray(b_attn[0 * C:1 * C][None, cols]),
            "bk": np.ascontiguousarray(b_attn[1 * C:2 * C][None, cols]),
            "bv": np.ascontiguousarray(b_attn[2 * C:3 * C][None, cols]),
            "wp": w_proj,
            "bp": bp,
            "ones512": ones512,
            "ropeC": ropeC,
            "ropeS": ropeS,
        })

    nc = _get_module()
    res = run_bass_kernel_spmd(nc, in_maps, core_ids=list(range(NCORES)),
                               trace=_trace)
    # core c returns [2*RPB, C]: batch-0 rows [RPB*c, RPB*(c+1)), then the
    # same rows of batch 1
    out = np.empty((B, T, C), dtype=FP)
    for c in range(NCORES):
        o = res.results[c]["out"]
        for b in range(B):
            out[b, RPB * c:RPB * (c + 1), :] = o[b * RPB:(b + 1) * RPB]
    if _trace:
        kernel.last_results = res
    return out



# revision 10
# speedup vs baseline: 1.3437x; 1.3437x over previous
"""Causal self-attention (fused QKV + RoPE + causal softmax + out-proj) on 8
Trainium2 NeuronCores.

Sharding: tensor-parallel by heads. 16 heads / 8 cores = 2 heads per core.
Each core computes q/k/v projections for its 2 heads over the full sequence
(column-parallel c_attn), RoPE, causal attention, producing y^T for its head
slice. Two per-batch AllToAlls reshard y from head-split to row-split (the
first batch's exchange overlaps the second batch's attention), after which
each core computes the output projection for its 2x256 rows (row-parallel
c_proj without an allreduce: the A2A moves head channels, not partial sums).

Device layout notes:
  - All matmul operands are bf16 (PSUM accumulation stays fp32).
  - TRN2 PE: K<128 (partial row-group) LDWEIGHTS+MATMUL chains serialize
    (~930ns per 512-col matmul) and never warm the HAM clock gate; full-K
    chains pipeline (~256ns) and reach 2.4GHz. So the per-head S^T matmuls
    (head_dim=64 contraction) are issued as K=128 matmuls against the
    2-head kT stationary, with the moving q operand ZERO-PADDED in the
    other head's 64 partitions (q0T rows 64:128 = 0, q1T rows 0:64 = 0).
  - S is computed transposed (S^T[k, q] = kT.T @ qT) flash-style per key
    block, so softmax normalizers come for free from an augmented V matmul
    ([V | 1]) and no P^T transpose is needed for the A*V matmul.
  - QKV biases ride inside the rope scalar_tensor_tensor ops (q/k) and the
    V psum->sbuf copy (ACT Identity+bias) instead of K=1 PE matmuls.
  - Softmax skips max-subtraction: |S|max ~ 6.5 for this problem, exp is
    safely in fp32 range.
  - Batch 1 is processed FIRST in attention so its (slow, ~25us) AllToAll
    overlaps batch 0's attention; batch 0's A2A is hidden under proj(b=1).
"""

import sys

sys.path.insert(0, "/opt/trn_rl_repo")

import ml_dtypes
import numpy as np

import concourse.bass as bass
import concourse.mybir as mybir
import concourse.tile as tile
from concourse import bacc
from concourse.bass_utils import run_bass_kernel_spmd
from concourse.masks import make_identity

B, T, C = 2, 2048, 1024
H, HD = 16, 64
HALF = HD // 2  # 32
NCORES = 8
HPC = H // NCORES  # 2 heads per core
ROWS = B * T  # 4096
DH = HPC * HD  # 128 channels per core
RPB = T // NCORES  # 256 rows per (core, batch)
ROPE_BASE = 10000.0
DT = mybir.dt.float32
MDT = mybir.dt.bfloat16  # matmul operand dtype
FP = np.float32
BF = ml_dtypes.bfloat16

KB = T // 128  # 16 key blocks per batch
QCH = 1024  # attention strip chunk width

# attention processes batches in this order; the first one's A2A overlaps
# the second one's attention compute
BORDER = (1, 0)

# debug: when True, the module dumps qT/kT/yT per core as extra outputs
DEBUG_TAPS = False


def _build_module(use_collective=True):
    assert HPC == 2
    nc = bacc.Bacc("TRN2", target_bir_lowering=False, debug=False,
                   num_devices=NCORES)

    xT_t = nc.dram_tensor("xT", [C, ROWS], MDT, kind="ExternalInput")
    wq_t = nc.dram_tensor("wq", [C, DH], MDT, kind="ExternalInput")
    wk_t = nc.dram_tensor("wk", [C, DH], MDT, kind="ExternalInput")
    wv_t = nc.dram_tensor("wv", [C, DH], MDT, kind="ExternalInput")
    bqc_t = nc.dram_tensor("bqc", [DH, 1], DT, kind="ExternalInput")
    bkc_t = nc.dram_tensor("bkc", [DH, 1], DT, kind="ExternalInput")
    bvc_t = nc.dram_tensor("bvc", [DH, 1], DT, kind="ExternalInput")
    wp_t = nc.dram_tensor("wp", [C, C], MDT, kind="ExternalInput")
    bp_t = nc.dram_tensor("bp", [1, C], MDT, kind="ExternalInput")
    ones_t = nc.dram_tensor("ones512", [1, 512], MDT, kind="ExternalInput")
    ropeC_t = nc.dram_tensor("ropeC", [DH, ROWS], DT, kind="ExternalInput")
    ropeS_t = nc.dram_tensor("ropeS", [DH, ROWS], DT, kind="ExternalInput")
    # rows 0:RPB = batch-0 rows [RPB*c, RPB*(c+1)), rows RPB: = batch-1 same
    out_t = nc.dram_tensor("out", [2 * RPB, C], DT, kind="ExternalOutput")
    if DEBUG_TAPS:
        qT_dbg = nc.dram_tensor("qT_dbg", [DH, ROWS], MDT,
                                kind="ExternalOutput")
        kT_dbg = nc.dram_tensor("kT_dbg", [DH, ROWS], MDT,
                                kind="ExternalOutput")
        yT_dbg = nc.dram_tensor("yT_dbg", [DH, ROWS], MDT,
                                kind="ExternalOutput")

    NCI = C // 128  # 8 contraction chunks

    with tile.TileContext(nc) as tc:
        with (
            tc.tile_pool(name="persist", bufs=1) as pp,
            tc.tile_pool(name="dram", bufs=1, space="DRAM") as dp,
        ):
            ident = pp.tile([128, 64], MDT, tag="ident")
            make_identity(nc, ident[0:64, :])
            nc.vector.tensor_copy(ident[64:128, :], ident[0:64, :])
            ones_row = pp.tile([1, 512], MDT, tag="ones_row")
            bqc = pp.tile([DH, 1], DT, tag="bqc")
            bkc = pp.tile([DH, 1], DT, tag="bkc")
            bvc = pp.tile([DH, 1], DT, tag="bvc")

            # per-batch A2A buffers: shard j of a2a_in[b] -> core j, which
            # then holds all head channels for batch-b rows [RPB*j, RPB*j+RPB)
            a2a_in = [dp.tile([NCORES, DH, RPB], MDT, tag=f"a2a_in{b}",
                              name=f"a2a_in{b}") for b in range(B)]
            a2a_out = [dp.tile([NCORES, DH, RPB], MDT, tag=f"a2a_out{b}",
                               name=f"a2a_out{b}") for b in range(B)]

            wp_sb = [pp.tile([128, C], MDT, tag=f"wp{ci}", name=f"wp{ci}")
                     for ci in range(NCI)]
            bp = pp.tile([1, C], MDT, tag="bp")
            yr = [pp.tile([128, NCORES, RPB], MDT, tag=f"yr{b}",
                          name=f"yr{b}") for b in range(B)]

            with tc.tile_pool(name="p12", bufs=1) as p12:
                # per-head zero-padded q (full-K moving operand for S)
                q0T = p12.tile([128, ROWS], MDT, tag="q0T")
                q1T = p12.tile([128, ROWS], MDT, tag="q1T")
                nc.vector.memset(q0T[64:128, :], 0.0)
                nc.vector.memset(q1T[0:64, :], 0.0)
                qTs = (q0T, q1T)
                kT = p12.tile([DH, ROWS], MDT, tag="kT")
                # V_all[:, (tb*2 + h), 0:64] = V rows for global 128-token
                # block tb, head h; col 64 = ones (softmax denominator).
                V_all = p12.tile([128, 2 * KB * HPC, HD + 1], MDT, tag="V_all")
                nc.vector.memset(V_all[:, :, HD:HD + 1], 1.0)
                yT = p12.tile([DH, ROWS], MDT, tag="yT")

                # ---------------- phase 1: qkv projection + rope ----------
                with (
                    nc.named_scope("qkv"),
                    tc.tile_pool(name="ph1", bufs=1) as ph1,
                    tc.tile_pool(name="ph1ps", bufs=1, space="PSUM") as ph1ps,
                ):
                    wq_sb = [ph1.tile([128, DH], MDT, tag=f"wq{ci}", name=f"wq{ci}") for ci in range(NCI)]
                    wk_sb = [ph1.tile([128, DH], MDT, tag=f"wk{ci}", name=f"wk{ci}") for ci in range(NCI)]
                    wv_sb = [ph1.tile([128, DH], MDT, tag=f"wv{ci}", name=f"wv{ci}") for ci in range(NCI)]
                    # rope tables on the scalar HWDGE ring so they don't
                    # delay the first xq chunk on the sync ring
                    C_sb = ph1.tile([DH, ROWS], DT, tag="ropeC")
                    S_sb = ph1.tile([DH, ROWS], DT, tag="ropeS")
                    nc.scalar.dma_start(C_sb[:], ropeC_t[:])
                    nc.scalar.dma_start(S_sb[:], ropeS_t[:])

                    PART = [1, 0, 3, 2]  # rope half-rotation partner groups
                    P1C = 512  # phase-1 chunk width (1 PSUM bank)
                    for Q in range(ROWS // P1C):
                        cols = slice(Q * P1C, (Q + 1) * P1C)
                        qps = ph1ps.tile([128, P1C], DT, tag="qps", bufs=2)
                        kps = ph1ps.tile([128, P1C], DT, tag="kps", bufs=2)
                        vps = ph1ps.tile([128, P1C], DT, tag="vps", bufs=2)
                        with tc.tile_pool(name=f"xq{Q}", bufs=4) as xqp:
                            for ci in range(NCI):
                                if Q == 0:
                                    # weight loads interleaved with first use
                                    sl = slice(ci * 128, (ci + 1) * 128)
                                    nc.sync.dma_start(wq_sb[ci][:], wq_t[sl, :])
                                    nc.sync.dma_start(wk_sb[ci][:], wk_t[sl, :])
                                    nc.sync.dma_start(wv_sb[ci][:], wv_t[sl, :])
                                xq = xqp.tile([128, P1C], MDT, tag="xq")
                                nc.sync.dma_start(
                                    xq[:], xT_t[ci * 128:(ci + 1) * 128, cols])
                                if Q == 0 and ci == 0:
                                    # small constants: after the critical
                                    # first weight/xq loads
                                    nc.sync.dma_start(ones_row[:], ones_t[:])
                                    nc.sync.dma_start(bqc[:], bqc_t[:])
                                    nc.sync.dma_start(bkc[:], bkc_t[:])
                                    nc.sync.dma_start(bvc[:], bvc_t[:])
                                st = ci == 0
                                sp = ci == NCI - 1
                                nc.tensor.matmul(qps[:], wq_sb[ci][:], xq[:],
                                                 start=st, stop=sp)
                                nc.tensor.matmul(kps[:], wk_sb[ci][:], xq[:],
                                                 start=st, stop=sp)
                                nc.tensor.matmul(vps[:], wv_sb[ci][:], xq[:],
                                                 start=st, stop=sp)

                        # rope: out = (q+b)*C + rot32(q+b)*S, fused with
                        # psum->sbuf eviction; bias rides in the DVE op
                        for ps_tile, bias_c, dsts in (
                                (qps, bqc, None), (kps, bkc, kT)):
                            ta = ph1.tile([128, P1C], DT, tag="ta", bufs=2)
                            tb_ = ph1.tile([128, P1C], DT, tag="tb", bufs=2)
                            # bias in place (PSUM) so the rope ops can keep
                            # their cross-partition operand slices
                            nc.vector.tensor_scalar(
                                ps_tile[:], ps_tile[:], bias_c[:], None,
                                mybir.AluOpType.add)
                            nc.vector.tensor_tensor(
                                ta[:], ps_tile[:], C_sb[:, cols],
                                mybir.AluOpType.mult)
                            for g in range(4):
                                gs = slice(32 * g, 32 * g + 32)
                                prt = slice(32 * PART[g], 32 * PART[g] + 32)
                                nc.vector.tensor_tensor(
                                    tb_[gs, :], ps_tile[prt, :], S_sb[gs, cols],
                                    mybir.AluOpType.mult)
                            if dsts is None:
                                nc.gpsimd.tensor_tensor(
                                    q0T[0:64, cols], ta[0:64, :], tb_[0:64, :],
                                    mybir.AluOpType.add)
                                nc.gpsimd.tensor_tensor(
                                    q1T[64:128, cols], ta[64:128, :],
                                    tb_[64:128, :], mybir.AluOpType.add)
                            else:
                                nc.gpsimd.tensor_tensor(
                                    dsts[:, cols], ta[:], tb_[:],
                                    mybir.AluOpType.add)

                        # V: bias-add copy out (ACT) then transpose into V_all
                        vt = ph1.tile([128, P1C], MDT, tag="vt", bufs=2)
                        nc.scalar.activation(
                            vt[:], vps[:],
                            mybir.ActivationFunctionType.Identity,
                            bias=bvc[:])
                        for tb in range(P1C // 128):
                            gtb = Q * (P1C // 128) + tb  # global 128-token blk
                            for h in range(HPC):
                                vap = ph1ps.tile([128, HD], MDT, tag="vap",
                                                 bufs=2)
                                nc.tensor.transpose(
                                    vap[:],
                                    vt[h * HD:(h + 1) * HD,
                                       tb * 128:(tb + 1) * 128],
                                    ident[h * HD:(h + 1) * HD, :])
                                nc.scalar.copy(
                                    V_all[:, gtb * HPC + h, 0:HD], vap[:])

                # load w_proj during attention (off the startup critical path)
                for ci in range(NCI):
                    nc.sync.dma_start(wp_sb[ci][:],
                                      wp_t[ci * 128:(ci + 1) * 128, :])
                nc.sync.dma_start(bp[:], bp_t[:])

                # ---------------- phase 2: causal attention ---------------
                # Software-pipelined: `delayed` holds closures (prev chunk's
                # A*V matmuls, unit finalizes, per-batch A2A staging) that are
                # drained one per chunk so ACT/DVE work hides under PE.
                with (
                    nc.named_scope("attn"),
                    tc.tile_pool(name="ph2", bufs=1) as ph2,
                    tc.tile_pool(name="ph2ps", bufs=1, space="PSUM") as ph2ps,
                ):
                    from collections import deque
                    delayed = deque()

                    def drain_all():
                        while delayed:
                            delayed.popleft()()

                    def make_finalize(oacc_u, h_u, bT_u, qb_u):
                        hp_u = slice(h_u * HD, (h_u + 1) * HD)

                        def fin():
                            linv = ph2.tile([1, 512], DT, tag="linv",
                                            bufs=3, name="linv")
                            # recip_approx (custom DVE) misreads PSUM inputs
                            # on HW: stage the denominator row in SBUF first
                            dsb = ph2.tile([1, 512], DT, tag="dsb",
                                           bufs=3, name="dsb")
                            nc.vector.tensor_copy(
                                dsb[:], oacc_u[qb_u][HD:HD + 1, :])
                            nc.vector.reciprocal_approx_fast(linv[:], dsb[:])
                            rsb = ph2.tile([64, 512], DT, tag="rsb",
                                           bufs=3, name="rsb")
                            nc.gpsimd.partition_broadcast(
                                rsb[:], linv[:], channels=64)
                            nc.vector.tensor_tensor(
                                yT[hp_u, bT_u + qb_u * 512:
                                   bT_u + (qb_u + 1) * 512],
                                oacc_u[qb_u][0:HD, :], rsb[:],
                                mybir.AluOpType.mult)
                        return fin

                    def make_a2a(b_u, load_yr):
                        def stage():
                            for j in range(NCORES):
                                nc.sync.dma_start(
                                    a2a_in[b_u][j],
                                    yT[:, b_u * T + j * RPB:
                                       b_u * T + (j + 1) * RPB])
                            if use_collective:
                                nc.gpsimd.collective_compute(
                                    "AllToAll", mybir.AluOpType.bypass,
                                    replica_groups=[list(range(NCORES))],
                                    ins=[a2a_in[b_u].opt()],
                                    outs=[a2a_out[b_u].opt()])
                            else:
                                nc.sync.dma_start(a2a_out[b_u][:],
                                                  a2a_in[b_u][:])
                            if load_yr:
                                # the first-processed batch's yr is loaded
                                # here (its A2A completes under the second
                                # batch's attention); the other batch's yr
                                # is loaded mid-proj
                                for ci in range(NCORES):
                                    nc.sync.dma_start(yr[b_u][:, ci, :],
                                                      a2a_out[b_u][ci])
                        return stage

                    for bi, b in enumerate(BORDER):
                        for h in range(HPC):
                            hp = slice(h * HD, (h + 1) * HD)
                            bT = b * T
                            oacc = [ph2ps.tile([HD + 1, 512], DT,
                                               tag=f"oacc{qb}",
                                               name=f"oacc{qb}")
                                    for qb in range(T // 512)]
                            for kb in range(KB):
                                qs = kb * 128
                                # full-K stationary: both heads' kT rows; the
                                # moving q is zero in the other head's rows
                                lhs_k = kT[:, bT + qs:bT + qs + 128]
                                off = qs
                                while off < T:
                                    cw = min(QCH, T - off)
                                    qoff = off
                                    off += cw
                                    sps = ph2ps.tile([128, QCH], DT,
                                                     tag="strip", bufs=2,
                                                     name="sps")
                                    for po in range(0, cw, 512):
                                        w = min(512, cw - po)
                                        nc.tensor.matmul(
                                            sps[:, po:po + w], lhs_k,
                                            qTs[h][:, bT + qoff + po:
                                                   bT + qoff + po + w],
                                            start=True, stop=True)
                                    # bounded 2-deep pipeline: spread fin
                                    # bursts across a couple of chunks
                                    while len(delayed) > 2:
                                        delayed.popleft()()
                                    psb = ph2.tile([128, QCH], MDT, tag="psb",
                                                   bufs=4, name="psb")
                                    nc.scalar.activation(
                                        psb[:, 0:cw], sps[:, 0:cw],
                                        mybir.ActivationFunctionType.Exp,
                                        scale=1.0 / float(np.sqrt(HD)))
                                    if qoff == qs:
                                        # zero strict upper triangle (k > q)
                                        nc.gpsimd.affine_select(
                                            out=psb[:, 0:128],
                                            in_=psb[:, 0:128],
                                            compare_op=mybir.AluOpType.is_ge,
                                            fill=0.0, base=0,
                                            pattern=[[1, 128]],
                                            channel_multiplier=-1)

                                    def make_av(oacc_u=oacc, psb_u=psb,
                                                kb_u=kb, qoff_u=qoff,
                                                cw_u=cw, b_u=b, h_u=h):
                                        def av():
                                            vidx = ((b_u * KB + kb_u) * HPC
                                                    + h_u)
                                            for qb in range(T // 512):
                                                lo = max(qoff_u, qb * 512)
                                                hi = min(qoff_u + cw_u,
                                                         qb * 512 + 512)
                                                if lo >= hi:
                                                    continue
                                                nc.tensor.matmul(
                                                    oacc_u[qb][:,
                                                               lo - qb * 512:
                                                               hi - qb * 512],
                                                    V_all[:, vidx, :],
                                                    psb_u[:, lo - qoff_u:
                                                          hi - qoff_u],
                                                    start=(kb_u == 0),
                                                    stop=(kb_u == 4 * qb + 3))
                                        return av

                                    delayed.append(make_av())
                            for qb in range(T // 512):
                                delayed.append(make_finalize(oacc, h, bT, qb))
                        delayed.append(make_a2a(b, load_yr=(bi == 0)))
                    drain_all()
                    if DEBUG_TAPS:
                        nc.sync.dma_start(qT_dbg[0:64, :], q0T[0:64, :])
                        nc.sync.dma_start(qT_dbg[64:128, :], q1T[64:128, :])
                        nc.sync.dma_start(kT_dbg[:], kT[:])
                        nc.sync.dma_start(yT_dbg[:], yT[:])

            # ---------------- phase 3: output projection ------------------
            with (
                nc.named_scope("proj"),
                tc.tile_pool(name="ph3", bufs=1) as ph3,
                tc.tile_pool(name="ph3ps", bufs=2, space="PSUM") as ph3ps,
            ):
                for bi, b in enumerate(BORDER):
                    if bi == 1:
                        for ci in range(NCORES):
                            nc.sync.dma_start(yr[b][:, ci, :], a2a_out[b][ci])
                    for tb in range(RPB // 128):
                        for co in range(C // 512):
                            pps = ph3ps.tile([128, 512], DT, tag="pps",
                                             name="pps")
                            for ci in range(NCI):
                                nc.tensor.matmul(
                                    pps[:],
                                    yr[b][:, ci, tb * 128:(tb + 1) * 128],
                                    wp_sb[ci][:, co * 512:(co + 1) * 512],
                                    start=(ci == 0), stop=False)
                            nc.tensor.matmul(pps[:], ones_row[:, 0:128],
                                             bp[:, co * 512:(co + 1) * 512],
                                             start=False, stop=True)
                            osb = ph3.tile([128, 512], DT, tag="osb", bufs=2,
                                           name="osb")
                            nc.vector.tensor_copy(osb[:], pps[:])
                            nc.sync.dma_start(
                                out_t[b * RPB + tb * 128:
                                      b * RPB + (tb + 1) * 128,
                                      co * 512:(co + 1) * 512], osb[:])

    nc.compile()
    return nc


_NC_CACHE = None


def _get_module():
    global _NC_CACHE
    if _NC_CACHE is None:
        _NC_CACHE = _build_module()
    return _NC_CACHE


def _rope_tables():
    inv = ROPE_BASE ** (-np.arange(HALF, dtype=np.float64) / HALF)
    tt = np.arange(T, dtype=np.float64)
    ang = tt[None, :] * inv[:, None]  # [32, T]
    cos = np.cos(ang).astype(FP)  # [32, T]
    sin = np.sin(ang).astype(FP)
    Cq = np.concatenate([cos, cos], axis=0)  # [64, T] (p%32 freq)
    Sq = np.concatenate([-sin, sin], axis=0)
    # duplicate for the HPC heads (partition dim) and tile across B along
    # columns (t_global = b*T + tt)
    Cq = np.tile(Cq, (HPC, B))
    Sq = np.tile(Sq, (HPC, B))
    return np.ascontiguousarray(Cq), np.ascontiguousarray(Sq)


def kernel(x, w_attn, b_attn, w_proj, b_proj, _trace=False):
    x = np.asarray(x, dtype=FP)
    w_attn = np.asarray(w_attn, dtype=FP)
    b_attn = np.asarray(b_attn, dtype=FP)
    w_proj = np.asarray(w_proj, dtype=FP)
    b_proj = np.asarray(b_proj, dtype=FP)

    xT = np.ascontiguousarray(x.reshape(ROWS, C).T).astype(BF)  # [C, ROWS]
    ropeC, ropeS = _rope_tables()
    bp = np.ascontiguousarray(b_proj[None, :]).astype(BF)
    ones512 = np.ones((1, 512), BF)
    wp_bf = w_proj.astype(BF)

    in_maps = []
    for c in range(NCORES):
        h0 = HPC * c
        cols = slice(h0 * HD, (h0 + HPC) * HD)  # this core's head channels
        in_maps.append({
            "xT": xT,
            "wq": np.ascontiguousarray(w_attn[:, 0 * C:1 * C][:, cols]).astype(BF),
            "wk": np.ascontiguousarray(w_attn[:, 1 * C:2 * C][:, cols]).astype(BF),
            "wv": np.ascontiguousarray(w_attn[:, 2 * C:3 * C][:, cols]).astype(BF),
            "bqc": np.ascontiguousarray(b_attn[0 * C:1 * C][cols][:, None]),
            "bkc": np.ascontiguousarray(b_attn[1 * C:2 * C][cols][:, None]),
            "bvc": np.ascontiguousarray(b_attn[2 * C:3 * C][cols][:, None]),
            "wp": wp_bf,
            "bp": bp,
            "ones512": ones512,
            "ropeC": ropeC,
            "ropeS": ropeS,
        })

    nc = _get_module()
    res = run_bass_kernel_spmd(nc, in_maps, core_ids=list(range(NCORES)),
                               trace=_trace)
    # core c returns [2*RPB, C]: batch-0 rows [RPB*c, RPB*(c+1)), then the
    # same rows of batch 1
    out = np.empty((B, T, C), dtype=FP)
    for c in range(NCORES):
        o = res.results[c]["out"]
        for b in range(B):
            out[b, RPB * c:RPB * (c + 1), :] = o[b * RPB:(b + 1) * RPB]
    kernel.last_results = res
    return out


# revision 13
# speedup vs baseline: 1.5204x; 1.1315x over previous
"""Causal self-attention (fused QKV + RoPE + causal softmax + out-proj) on 8
Trainium2 NeuronCores.

Sharding: tensor-parallel by heads. 16 heads / 8 cores = 2 heads per core.
Each core computes q/k/v projections for its 2 heads over the full sequence
(column-parallel c_attn), RoPE, causal attention, producing y^T for its head
slice. Four per-(batch,head) AllToAlls reshard y from head-split to
row-split (all but the last overlap attention compute), after which each
core computes the output projection for its 2x256 rows (row-parallel c_proj
without an allreduce: the A2A moves head channels, not partial sums).

Device layout notes:
  - All matmul operands are bf16 (PSUM accumulation stays fp32).
  - TRN2 PE: K<128 (partial row-group) LDWEIGHTS+MATMUL chains serialize
    (~930ns per 512-col matmul) and never warm the HAM clock gate; full-K
    chains pipeline (~256ns) and reach 2.4GHz. So the per-head S^T matmuls
    (head_dim=64 contraction) are issued as K=128 matmuls against the
    2-head kT stationary, with the moving q operand ZERO-PADDED in the
    other head's 64 partitions (q0T rows 64:128 = 0, q1T rows 0:64 = 0).
  - S is computed transposed (S^T[k, q] = kT.T @ qT) flash-style per key
    block, so softmax normalizers come for free from an augmented V matmul
    ([V | 1]) and no P^T transpose is needed for the A*V matmul.
  - Phase-1 x loads are spread across 4 engine DMA rings (sync/scalar/
    vector/gpsimd); a single ring caps at ~100GB/s and gates the phase.
  - Softmax skips max-subtraction: |S|max ~ 6.5 for this problem, exp is
    safely in fp32 range.
  - Batch 1 is processed FIRST in attention so its AllToAlls overlap batch
    0's attention; batch 0 head 0's A2A overlaps head 1's attention, and
    only head 1's (0.25MB) A2A lands at the end, hidden under proj(b=1).
"""

import sys

sys.path.insert(0, "/opt/trn_rl_repo")

import ml_dtypes
import numpy as np

import concourse.bass as bass
import concourse.mybir as mybir
import concourse.tile as tile
from concourse import bacc
from concourse.bass_utils import run_bass_kernel_spmd
from concourse.masks import make_identity

B, T, C = 2, 2048, 1024
H, HD = 16, 64
HALF = HD // 2  # 32
NCORES = 8
HPC = H // NCORES  # 2 heads per core
ROWS = B * T  # 4096
DH = HPC * HD  # 128 channels per core
RPB = T // NCORES  # 256 rows per (core, batch)
ROPE_BASE = 10000.0
DT = mybir.dt.float32
MDT = mybir.dt.bfloat16  # matmul operand dtype
FP = np.float32
BF = ml_dtypes.bfloat16

KB = T // 128  # 16 key blocks per batch
QCH = 1024  # attention strip chunk width

# attention processes batches in this order; the first one's A2As overlap
# the second one's attention compute
BORDER = (1, 0)

# debug: when True, the module dumps qT/kT/yT per core as extra outputs
DEBUG_TAPS = False


def _build_module(with_bias, use_collective=True):
    assert HPC == 2
    nc = bacc.Bacc("TRN2", target_bir_lowering=False, debug=False,
                   num_devices=NCORES)

    xT_t = nc.dram_tensor("xT", [C, ROWS], MDT, kind="ExternalInput")
    wq_t = nc.dram_tensor("wq", [C, DH], MDT, kind="ExternalInput")
    wk_t = nc.dram_tensor("wk", [C, DH], MDT, kind="ExternalInput")
    wv_t = nc.dram_tensor("wv", [C, DH], MDT, kind="ExternalInput")
    if with_bias:
        bqc_t = nc.dram_tensor("bqc", [DH, 1], DT, kind="ExternalInput")
        bkc_t = nc.dram_tensor("bkc", [DH, 1], DT, kind="ExternalInput")
        bvc_t = nc.dram_tensor("bvc", [DH, 1], DT, kind="ExternalInput")
    wp_t = nc.dram_tensor("wp", [C, C], MDT, kind="ExternalInput")
    bp_t = nc.dram_tensor("bp", [1, C], MDT, kind="ExternalInput")
    ones_t = nc.dram_tensor("ones512", [1, 512], MDT, kind="ExternalInput")
    # rope tables for ONE batch span [DH, T]; reused across batches
    ropeC_t = nc.dram_tensor("ropeC", [DH, T], DT, kind="ExternalInput")
    ropeS_t = nc.dram_tensor("ropeS", [DH, T], DT, kind="ExternalInput")
    # rows 0:RPB = batch-0 rows [RPB*c, RPB*(c+1)), rows RPB: = batch-1 same
    out_t = nc.dram_tensor("out", [2 * RPB, C], DT, kind="ExternalOutput")
    if DEBUG_TAPS:
        qT_dbg = nc.dram_tensor("qT_dbg", [DH, ROWS], MDT,
                                kind="ExternalOutput")
        kT_dbg = nc.dram_tensor("kT_dbg", [DH, ROWS], MDT,
                                kind="ExternalOutput")
        yT_dbg = nc.dram_tensor("yT_dbg", [DH, ROWS], MDT,
                                kind="ExternalOutput")

    NCI = C // 128  # 8 contraction chunks
    RINGS = None  # set inside context

    with tile.TileContext(nc) as tc:
        RINGS = (nc.sync, nc.scalar, nc.gpsimd)
        with (
            tc.tile_pool(name="persist", bufs=1) as pp,
            tc.tile_pool(name="dram", bufs=1, space="DRAM") as dp,
        ):
            ident = pp.tile([128, 64], MDT, tag="ident")
            make_identity(nc, ident[0:64, :])
            nc.vector.tensor_copy(ident[64:128, :], ident[0:64, :])
            ones_row = pp.tile([1, 512], MDT, tag="ones_row")
            if with_bias:
                bqc = pp.tile([DH, 1], DT, tag="bqc")
                bkc = pp.tile([DH, 1], DT, tag="bkc")
                bvc = pp.tile([DH, 1], DT, tag="bvc")

            # per-(batch,head) A2A buffers: shard j -> core j's 64 channels
            # of head h for batch-b rows [RPB*j, RPB*j+RPB)
            a2a_in = [[dp.tile([NCORES, HD, RPB], MDT, tag=f"a2a_in{b}{h}",
                               name=f"a2a_in{b}{h}") for h in range(HPC)]
                      for b in range(B)]
            a2a_out = [[dp.tile([NCORES, HD, RPB], MDT, tag=f"a2a_out{b}{h}",
                                name=f"a2a_out{b}{h}") for h in range(HPC)]
                       for b in range(B)]

            wp_sb = [pp.tile([128, C], MDT, tag=f"wp{ci}", name=f"wp{ci}")
                     for ci in range(NCI)]
            bp = pp.tile([1, C], MDT, tag="bp")
            yr = [pp.tile([128, NCORES, RPB], MDT, tag=f"yr{b}",
                          name=f"yr{b}") for b in range(B)]

            with tc.tile_pool(name="p12", bufs=1) as p12:
                # per-head zero-padded q (full-K moving operand for S)
                q0T = p12.tile([128, ROWS], MDT, tag="q0T")
                q1T = p12.tile([128, ROWS], MDT, tag="q1T")
                nc.vector.memset(q0T[64:128, :], 0.0)
                nc.vector.memset(q1T[0:64, :], 0.0)
                qTs = (q0T, q1T)
                kT = p12.tile([DH, ROWS], MDT, tag="kT")
                # V_all[:, (tb*2 + h), 0:64] = V rows for global 128-token
                # block tb, head h; col 64 = ones (softmax denominator).
                V_all = p12.tile([128, 2 * KB * HPC, HD + 1], MDT, tag="V_all")
                nc.vector.memset(V_all[:, :, HD:HD + 1], 1.0)
                yT = p12.tile([DH, ROWS], MDT, tag="yT")

                # ---------------- phase 1: qkv projection + rope ----------
                with (
                    nc.named_scope("qkv"),
                    tc.tile_pool(name="ph1", bufs=1) as ph1,
                    tc.tile_pool(name="ph1ps", bufs=1, space="PSUM") as ph1ps,
                ):
                    wq_sb = [ph1.tile([128, DH], MDT, tag=f"wq{ci}", name=f"wq{ci}") for ci in range(NCI)]
                    wk_sb = [ph1.tile([128, DH], MDT, tag=f"wk{ci}", name=f"wk{ci}") for ci in range(NCI)]
                    wv_sb = [ph1.tile([128, DH], MDT, tag=f"wv{ci}", name=f"wv{ci}") for ci in range(NCI)]
                    C_sb = ph1.tile([DH, T], DT, tag="ropeC")
                    S_sb = ph1.tile([DH, T], DT, tag="ropeS")

                    PART = [1, 0, 3, 2]  # rope half-rotation partner groups
                    P1C = 512  # phase-1 psum chunk width (1 PSUM bank)
                    XW = 1024  # xq tile width
                    for QB in range(ROWS // XW):
                        bcols = slice(QB * XW, (QB + 1) * XW)
                        tcol = (QB * XW) % T  # column in the rope table
                        if QB * XW < T:
                            # rope table slices just-in-time (scalar ring)
                            nc.scalar.dma_start(
                                C_sb[:, QB * XW:(QB + 1) * XW],
                                ropeC_t[:, QB * XW:(QB + 1) * XW])
                            nc.scalar.dma_start(
                                S_sb[:, QB * XW:(QB + 1) * XW],
                                ropeS_t[:, QB * XW:(QB + 1) * XW])
                        xqs = []
                        for ci in range(NCI):
                            ring = RINGS[ci % 3]
                            if QB == 0:
                                # weight loads interleaved with first use
                                sl = slice(ci * 128, (ci + 1) * 128)
                                ring.dma_start(wq_sb[ci][:], wq_t[sl, :])
                                ring.dma_start(wk_sb[ci][:], wk_t[sl, :])
                                ring.dma_start(wv_sb[ci][:], wv_t[sl, :])
                            xq = ph1.tile([128, XW], MDT, tag=f"xq{ci}",
                                          bufs=2, name=f"xq{ci}")
                            ring.dma_start(
                                xq[:], xT_t[ci * 128:(ci + 1) * 128, bcols])
                            xqs.append(xq)
                            if QB == 0 and ci == 0:
                                nc.sync.dma_start(ones_row[:], ones_t[:])
                                if with_bias:
                                    nc.sync.dma_start(bqc[:], bqc_t[:])
                                    nc.sync.dma_start(bkc[:], bkc_t[:])
                                    nc.sync.dma_start(bvc[:], bvc_t[:])

                        for hf in range(XW // P1C):
                            hs = slice(hf * P1C, (hf + 1) * P1C)
                            cols = slice(QB * XW + hf * P1C,
                                         QB * XW + (hf + 1) * P1C)
                            tcols = slice(tcol + hf * P1C,
                                          tcol + (hf + 1) * P1C)
                            qps = ph1ps.tile([128, P1C], DT, tag="qps",
                                             bufs=2)
                            kps = ph1ps.tile([128, P1C], DT, tag="kps",
                                             bufs=2)
                            vps = ph1ps.tile([128, P1C], DT, tag="vps",
                                             bufs=2)
                            for ci in range(NCI):
                                st = ci == 0
                                sp = ci == NCI - 1
                                nc.tensor.matmul(qps[:], wq_sb[ci][:],
                                                 xqs[ci][:, hs],
                                                 start=st, stop=sp)
                                nc.tensor.matmul(kps[:], wk_sb[ci][:],
                                                 xqs[ci][:, hs],
                                                 start=st, stop=sp)
                                nc.tensor.matmul(vps[:], wv_sb[ci][:],
                                                 xqs[ci][:, hs],
                                                 start=st, stop=sp)

                            # rope: out = (q+b)*C + rot32(q+b)*S, fused with
                            # psum->sbuf eviction
                            for ps_tile, bias_n, dsts in (
                                    (qps, "bqc", None), (kps, "bkc", kT)):
                                ta = ph1.tile([128, P1C], DT, tag="ta",
                                              bufs=2)
                                tb_ = ph1.tile([128, P1C], DT, tag="tb",
                                               bufs=2)
                                if with_bias:
                                    bias_c = bqc if bias_n == "bqc" else bkc
                                    nc.vector.tensor_scalar(
                                        ps_tile[:], ps_tile[:], bias_c[:],
                                        None, mybir.AluOpType.add)
                                nc.vector.tensor_tensor(
                                    ta[:], ps_tile[:], C_sb[:, tcols],
                                    mybir.AluOpType.mult)
                                for g in range(4):
                                    gs = slice(32 * g, 32 * g + 32)
                                    prt = slice(32 * PART[g],
                                                32 * PART[g] + 32)
                                    nc.vector.tensor_tensor(
                                        tb_[gs, :], ps_tile[prt, :],
                                        S_sb[gs, tcols],
                                        mybir.AluOpType.mult)
                                if dsts is None:
                                    nc.gpsimd.tensor_tensor(
                                        q0T[0:64, cols], ta[0:64, :],
                                        tb_[0:64, :], mybir.AluOpType.add)
                                    nc.gpsimd.tensor_tensor(
                                        q1T[64:128, cols], ta[64:128, :],
                                        tb_[64:128, :], mybir.AluOpType.add)
                                else:
                                    nc.gpsimd.tensor_tensor(
                                        dsts[:, cols], ta[:], tb_[:],
                                        mybir.AluOpType.add)

                            # V: copy out (ACT) then transpose into V_all
                            vt = ph1.tile([128, P1C], MDT, tag="vt", bufs=2)
                            if with_bias:
                                nc.scalar.activation(
                                    vt[:], vps[:],
                                    mybir.ActivationFunctionType.Identity,
                                    bias=bvc[:])
                            else:
                                nc.scalar.copy(vt[:], vps[:])
                            for tb in range(P1C // 128):
                                gtb = (QB * XW + hf * P1C) // 128 + tb
                                for h in range(HPC):
                                    vap = ph1ps.tile([128, HD], MDT,
                                                     tag="vap", bufs=2)
                                    nc.tensor.transpose(
                                        vap[:],
                                        vt[h * HD:(h + 1) * HD,
                                           tb * 128:(tb + 1) * 128],
                                        ident[h * HD:(h + 1) * HD, :])
                                    nc.scalar.copy(
                                        V_all[:, gtb * HPC + h, 0:HD],
                                        vap[:])

                # load w_proj during attention (off the startup critical path)
                for ci in range(NCI):
                    nc.sync.dma_start(wp_sb[ci][:],
                                      wp_t[ci * 128:(ci + 1) * 128, :])
                nc.sync.dma_start(bp[:], bp_t[:])

                # ---------------- phase 2: causal attention ---------------
                # Software-pipelined: `delayed` holds closures (prev chunk's
                # A*V matmuls, per-qb finalizes, per-(b,h) A2A staging) that
                # are drained one per chunk so ACT/DVE hides under PE.
                with (
                    nc.named_scope("attn"),
                    tc.tile_pool(name="ph2", bufs=1) as ph2,
                    tc.tile_pool(name="ph2ps", bufs=1, space="PSUM") as ph2ps,
                ):
                    from collections import deque
                    delayed = deque()

                    def drain_all():
                        while delayed:
                            delayed.popleft()()

                    def make_finalize(oacc_u, h_u, bT_u, qb_u):
                        hp_u = slice(h_u * HD, (h_u + 1) * HD)

                        def fin():
                            linv = ph2.tile([1, 512], DT, tag="linv",
                                            bufs=3, name="linv")
                            # recip_approx (custom DVE) misreads PSUM inputs
                            # on HW: stage the denominator in SBUF first
                            dsb = ph2.tile([1, 512], DT, tag="dsb",
                                           bufs=3, name="dsb")
                            nc.vector.tensor_copy(
                                dsb[:], oacc_u[qb_u][HD:HD + 1, :])
                            nc.vector.reciprocal_approx_fast(linv[:], dsb[:])
                            rsb = ph2.tile([64, 512], DT, tag="rsb",
                                           bufs=3, name="rsb")
                            nc.gpsimd.partition_broadcast(
                                rsb[:], linv[:], channels=64)
                            nc.vector.tensor_tensor(
                                yT[hp_u, bT_u + qb_u * 512:
                                   bT_u + (qb_u + 1) * 512],
                                oacc_u[qb_u][0:HD, :], rsb[:],
                                mybir.AluOpType.mult)
                        return fin

                    def make_a2a(b_u, h_u, load_yr):
                        hp_u = slice(h_u * HD, (h_u + 1) * HD)

                        def stage():
                            for j in range(NCORES):
                                nc.sync.dma_start(
                                    a2a_in[b_u][h_u][j],
                                    yT[hp_u, b_u * T + j * RPB:
                                       b_u * T + (j + 1) * RPB])
                            if use_collective:
                                nc.gpsimd.collective_compute(
                                    "AllToAll", mybir.AluOpType.bypass,
                                    replica_groups=[list(range(NCORES))],
                                    ins=[a2a_in[b_u][h_u].opt()],
                                    outs=[a2a_out[b_u][h_u].opt()])
                            else:
                                nc.sync.dma_start(a2a_out[b_u][h_u][:],
                                                  a2a_in[b_u][h_u][:])
                            if load_yr:
                                for ci in range(NCORES):
                                    nc.sync.dma_start(
                                        yr[b_u][h_u * HD:(h_u + 1) * HD,
                                                ci, :],
                                        a2a_out[b_u][h_u][ci])
                        return stage

                    for bi, b in enumerate(BORDER):
                        for h in range(HPC):
                            hp = slice(h * HD, (h + 1) * HD)
                            bT = b * T
                            oacc = [ph2ps.tile([HD + 1, 512], DT,
                                               tag=f"oacc{qb}",
                                               name=f"oacc{qb}")
                                    for qb in range(T // 512)]
                            for kb in range(KB):
                                qs = kb * 128
                                # full-K stationary: both heads' kT rows; the
                                # moving q is zero in the other head's rows
                                lhs_k = kT[:, bT + qs:bT + qs + 128]
                                off = qs
                                while off < T:
                                    cw = min(QCH, T - off)
                                    qoff = off
                                    off += cw
                                    sps = ph2ps.tile([128, QCH], DT,
                                                     tag="strip", bufs=2,
                                                     name="sps")
                                    for po in range(0, cw, 512):
                                        w = min(512, cw - po)
                                        nc.tensor.matmul(
                                            sps[:, po:po + w], lhs_k,
                                            qTs[h][:, bT + qoff + po:
                                                   bT + qoff + po + w],
                                            start=True, stop=True)
                                    # bounded 2-deep pipeline: spread fin
                                    # bursts across a couple of chunks
                                    while len(delayed) > 2:
                                        delayed.popleft()()
                                    psb = ph2.tile([128, QCH], MDT, tag="psb",
                                                   bufs=4, name="psb")
                                    nc.scalar.activation(
                                        psb[:, 0:cw], sps[:, 0:cw],
                                        mybir.ActivationFunctionType.Exp,
                                        scale=1.0 / float(np.sqrt(HD)))
                                    if qoff == qs:
                                        # zero strict upper triangle (k > q)
                                        nc.gpsimd.affine_select(
                                            out=psb[:, 0:128],
                                            in_=psb[:, 0:128],
                                            compare_op=mybir.AluOpType.is_ge,
                                            fill=0.0, base=0,
                                            pattern=[[1, 128]],
                                            channel_multiplier=-1)

                                    def make_av(oacc_u=oacc, psb_u=psb,
                                                kb_u=kb, qoff_u=qoff,
                                                cw_u=cw, b_u=b, h_u=h):
                                        def av():
                                            vidx = ((b_u * KB + kb_u) * HPC
                                                    + h_u)
                                            for qb in range(T // 512):
                                                lo = max(qoff_u, qb * 512)
                                                hi = min(qoff_u + cw_u,
                                                         qb * 512 + 512)
                                                if lo >= hi:
                                                    continue
                                                nc.tensor.matmul(
                                                    oacc_u[qb][:,
                                                               lo - qb * 512:
                                                               hi - qb * 512],
                                                    V_all[:, vidx, :],
                                                    psb_u[:, lo - qoff_u:
                                                          hi - qoff_u],
                                                    start=(kb_u == 0),
                                                    stop=(kb_u == 4 * qb + 3))
                                        return av

                                    delayed.append(make_av())
                                # early finalize: oacc[qb] stops accumulating
                                # at kb == 4*qb+3 (its stop lands in this
                                # strip's first chunk)
                                if kb % 4 == 3:
                                    delayed.append(
                                        make_finalize(oacc, h, bT, kb // 4))
                            delayed.append(
                                make_a2a(b, h, load_yr=(bi == 0)))
                    drain_all()
                    if DEBUG_TAPS:
                        nc.sync.dma_start(qT_dbg[0:64, :], q0T[0:64, :])
                        nc.sync.dma_start(qT_dbg[64:128, :], q1T[64:128, :])
                        nc.sync.dma_start(kT_dbg[:], kT[:])
                        nc.sync.dma_start(yT_dbg[:], yT[:])

            # ---------------- phase 3: output projection ------------------
            with (
                nc.named_scope("proj"),
                tc.tile_pool(name="ph3", bufs=1) as ph3,
                tc.tile_pool(name="ph3ps", bufs=2, space="PSUM") as ph3ps,
            ):
                for bi, b in enumerate(BORDER):
                    if bi == 1:
                        for h in range(HPC):
                            for ci in range(NCORES):
                                nc.sync.dma_start(
                                    yr[b][h * HD:(h + 1) * HD, ci, :],
                                    a2a_out[b][h][ci])
                    for tb in range(RPB // 128):
                        for co in range(C // 512):
                            pps = ph3ps.tile([128, 512], DT, tag="pps",
                                             name="pps")
                            for ci in range(NCI):
                                nc.tensor.matmul(
                                    pps[:],
                                    yr[b][:, ci, tb * 128:(tb + 1) * 128],
                                    wp_sb[ci][:, co * 512:(co + 1) * 512],
                                    start=(ci == 0), stop=False)
                            nc.tensor.matmul(pps[:], ones_row[:, 0:128],
                                             bp[:, co * 512:(co + 1) * 512],
                                             start=False, stop=True)
                            osb = ph3.tile([128, 512], DT, tag="osb", bufs=2,
                                           name="osb")
                            nc.vector.tensor_copy(osb[:], pps[:])
                            nc.sync.dma_start(
                                out_t[b * RPB + tb * 128:
                                      b * RPB + (tb + 1) * 128,
                                      co * 512:(co + 1) * 512], osb[:])

    nc.compile()
    return nc


_NC_CACHE = {}


def _get_module(with_bias):
    if with_bias not in _NC_CACHE:
        _NC_CACHE[with_bias] = _build_module(with_bias)
    return _NC_CACHE[with_bias]


def _rope_tables():
    inv = ROPE_BASE ** (-np.arange(HALF, dtype=np.float64) / HALF)
    tt = np.arange(T, dtype=np.float64)
    ang = tt[None, :] * inv[:, None]  # [32, T]
    cos = np.cos(ang).astype(FP)  # [32, T]
    sin = np.sin(ang).astype(FP)
    Cq = np.concatenate([cos, cos], axis=0)  # [64, T] (p%32 freq)
    Sq = np.concatenate([-sin, sin], axis=0)
    # duplicate for the HPC heads (partition dim); columns span one batch
    Cq = np.tile(Cq, (HPC, 1))
    Sq = np.tile(Sq, (HPC, 1))
    return np.ascontiguousarray(Cq), np.ascontiguousarray(Sq)


def kernel(x, w_attn, b_attn, w_proj, b_proj, _trace=False):
    x = np.asarray(x, dtype=FP)
    w_attn = np.asarray(w_attn, dtype=FP)
    b_attn = np.asarray(b_attn, dtype=FP)
    w_proj = np.asarray(w_proj, dtype=FP)
    b_proj = np.asarray(b_proj, dtype=FP)

    with_bias = bool(np.any(b_attn))  # q/k/v biases (bp always applied)

    xT = np.ascontiguousarray(x.reshape(ROWS, C).T).astype(BF)  # [C, ROWS]
    ropeC, ropeS = _rope_tables()
    bp = np.ascontiguousarray(b_proj[None, :]).astype(BF)
    ones512 = np.ones((1, 512), BF)
    wp_bf = w_proj.astype(BF)

    in_maps = []
    for c in range(NCORES):
        h0 = HPC * c
        cols = slice(h0 * HD, (h0 + HPC) * HD)  # this core's head channels
        m = {
            "xT": xT,
            "wq": np.ascontiguousarray(w_attn[:, 0 * C:1 * C][:, cols]).astype(BF),
            "wk": np.ascontiguousarray(w_attn[:, 1 * C:2 * C][:, cols]).astype(BF),
            "wv": np.ascontiguousarray(w_attn[:, 2 * C:3 * C][:, cols]).astype(BF),
            "wp": wp_bf,
            "bp": bp,
            "ones512": ones512,
            "ropeC": ropeC,
            "ropeS": ropeS,
        }
        if with_bias:
            m["bqc"] = np.ascontiguousarray(b_attn[0 * C:1 * C][cols][:, None])
            m["bkc"] = np.ascontiguousarray(b_attn[1 * C:2 * C][cols][:, None])
            m["bvc"] = np.ascontiguousarray(b_attn[2 * C:3 * C][cols][:, None])
        in_maps.append(m)

    nc = _get_module(with_bias)
    res = run_bass_kernel_spmd(nc, in_maps, core_ids=list(range(NCORES)),
                               trace=_trace)
    # core c returns [2*RPB, C]: batch-0 rows [RPB*c, RPB*(c+1)), then the
    # same rows of batch 1
    out = np.empty((B, T, C), dtype=FP)
    for c in range(NCORES):
        o = res.results[c]["out"]
        for b in range(B):
            out[b, RPB * c:RPB * (c + 1), :] = o[b * RPB:(b + 1) * RPB]
    kernel.last_results = res
    return out


# revision 15
# speedup vs baseline: 1.5864x; 1.0434x over previous
"""Causal self-attention (fused QKV + RoPE + causal softmax + out-proj) on 8
Trainium2 NeuronCores.

Sharding: tensor-parallel by heads. 16 heads / 8 cores = 2 heads per core.
Each core computes q/k/v projections for its 2 heads over the full sequence
(column-parallel c_attn), RoPE, causal attention, producing y^T for its head
slice. Four per-(batch,head) AllToAlls reshard y from head-split to
row-split (all but the last overlap attention compute), after which each
core computes the output projection for its 2x256 rows (row-parallel c_proj
without an allreduce: the A2A moves head channels, not partial sums).

Device layout notes:
  - All matmul operands are bf16 (PSUM accumulation stays fp32).
  - TRN2 PE: K<128 (partial row-group) LDWEIGHTS+MATMUL chains serialize
    (~930ns per 512-col matmul) and never warm the HAM clock gate; full-K
    chains pipeline (~256ns) and reach 2.4GHz. So the per-head S^T matmuls
    (head_dim=64 contraction) are issued as K=128 matmuls against the
    2-head kT stationary, with the moving q operand ZERO-PADDED in the
    other head's 64 partitions (q0T rows 64:128 = 0, q1T rows 0:64 = 0).
  - S is computed transposed (S^T[k, q] = kT.T @ qT) flash-style per key
    block, so softmax normalizers come for free from an augmented V matmul
    ([V | 1]) and no P^T transpose is needed for the A*V matmul.
  - Phase-1 x loads are spread across 4 engine DMA rings (sync/scalar/
    vector/gpsimd); a single ring caps at ~100GB/s and gates the phase.
  - Softmax skips max-subtraction: |S|max ~ 6.5 for this problem, exp is
    safely in fp32 range.
  - Batch 1 is processed FIRST in attention so its AllToAlls overlap batch
    0's attention; batch 0 head 0's A2A overlaps head 1's attention, and
    only head 1's (0.25MB) A2A lands at the end, hidden under proj(b=1).
"""

import sys

sys.path.insert(0, "/opt/trn_rl_repo")

import ml_dtypes
import numpy as np

import concourse.bass as bass
import concourse.mybir as mybir
import concourse.tile as tile
from concourse import bacc
from concourse.bass_utils import run_bass_kernel_spmd
from concourse.masks import make_identity

B, T, C = 2, 2048, 1024
H, HD = 16, 64
HALF = HD // 2  # 32
NCORES = 8
HPC = H // NCORES  # 2 heads per core
ROWS = B * T  # 4096
DH = HPC * HD  # 128 channels per core
RPB = T // NCORES  # 256 rows per (core, batch)
ROPE_BASE = 10000.0
DT = mybir.dt.float32
MDT = mybir.dt.bfloat16  # matmul operand dtype
FP = np.float32
BF = ml_dtypes.bfloat16

KB = T // 128  # 16 key blocks per batch
QCH = 1024  # attention strip chunk width

# attention processes batches in this order; the first one's A2As overlap
# the second one's attention compute
BORDER = (1, 0)

# debug: when True, the module dumps qT/kT/yT per core as extra outputs
DEBUG_TAPS = False


def _build_module(with_bias, use_collective=True):
    assert HPC == 2
    nc = bacc.Bacc("TRN2", target_bir_lowering=False, debug=False,
                   num_devices=NCORES)

    xT_t = nc.dram_tensor("xT", [C, ROWS], MDT, kind="ExternalInput")
    wq_t = nc.dram_tensor("wq", [C, DH], MDT, kind="ExternalInput")
    wk_t = nc.dram_tensor("wk", [C, DH], MDT, kind="ExternalInput")
    wv_t = nc.dram_tensor("wv", [C, DH], MDT, kind="ExternalInput")
    if with_bias:
        bqc_t = nc.dram_tensor("bqc", [DH, 1], DT, kind="ExternalInput")
        bkc_t = nc.dram_tensor("bkc", [DH, 1], DT, kind="ExternalInput")
        bvc_t = nc.dram_tensor("bvc", [DH, 1], DT, kind="ExternalInput")
    wp_t = nc.dram_tensor("wp", [C, C], MDT, kind="ExternalInput")
    bp_t = nc.dram_tensor("bp", [1, C], MDT, kind="ExternalInput")
    ones_t = nc.dram_tensor("ones512", [1, 512], MDT, kind="ExternalInput")
    # rope tables for ONE batch span [DH, T]; reused across batches
    ropeC_t = nc.dram_tensor("ropeC", [DH, T], MDT, kind="ExternalInput")
    ropeS_t = nc.dram_tensor("ropeS", [DH, T], MDT, kind="ExternalInput")
    # rows 0:RPB = batch-0 rows [RPB*c, RPB*(c+1)), rows RPB: = batch-1 same
    out_t = nc.dram_tensor("out", [2 * RPB, C], DT, kind="ExternalOutput")
    if DEBUG_TAPS:
        qT_dbg = nc.dram_tensor("qT_dbg", [DH, ROWS], MDT,
                                kind="ExternalOutput")
        kT_dbg = nc.dram_tensor("kT_dbg", [DH, ROWS], MDT,
                                kind="ExternalOutput")
        yT_dbg = nc.dram_tensor("yT_dbg", [DH, ROWS], MDT,
                                kind="ExternalOutput")

    NCI = C // 128  # 8 contraction chunks
    RINGS = None  # set inside context

    with tile.TileContext(nc) as tc:
        RINGS = (nc.sync, nc.scalar, nc.gpsimd)
        with (
            tc.tile_pool(name="persist", bufs=1) as pp,
            tc.tile_pool(name="dram", bufs=1, space="DRAM") as dp,
        ):
            ident = pp.tile([128, 64], MDT, tag="ident")
            make_identity(nc, ident[0:64, :])
            nc.vector.tensor_copy(ident[64:128, :], ident[0:64, :])
            # lower-triangle (k <= q) mask for the causal diagonal block
            maskD = pp.tile([128, 128], MDT, tag="maskD")
            nc.vector.memset(maskD[:], 1.0)
            nc.gpsimd.affine_select(
                out=maskD[:], in_=maskD[:],
                compare_op=mybir.AluOpType.is_ge, fill=0.0, base=0,
                pattern=[[1, 128]], channel_multiplier=-1)
            ones_row = pp.tile([1, 512], MDT, tag="ones_row")
            if with_bias:
                bqc = pp.tile([DH, 1], DT, tag="bqc")
                bkc = pp.tile([DH, 1], DT, tag="bkc")
                bvc = pp.tile([DH, 1], DT, tag="bvc")

            # per-(batch,head) A2A buffers: shard j -> core j's 64 channels
            # of head h for batch-b rows [RPB*j, RPB*j+RPB)
            a2a_in = [[dp.tile([NCORES, HD, RPB], MDT, tag=f"a2a_in{b}{h}",
                               name=f"a2a_in{b}{h}") for h in range(HPC)]
                      for b in range(B)]
            a2a_out = [[dp.tile([NCORES, HD, RPB], MDT, tag=f"a2a_out{b}{h}",
                                name=f"a2a_out{b}{h}") for h in range(HPC)]
                       for b in range(B)]

            wp_sb = [pp.tile([128, C], MDT, tag=f"wp{ci}", name=f"wp{ci}")
                     for ci in range(NCI)]
            bp = pp.tile([1, C], MDT, tag="bp")
            yr = [pp.tile([128, NCORES, RPB], MDT, tag=f"yr{b}",
                          name=f"yr{b}") for b in range(B)]

            with tc.tile_pool(name="p12", bufs=1) as p12:
                # per-head zero-padded q (full-K moving operand for S)
                q0T = p12.tile([128, ROWS], MDT, tag="q0T")
                q1T = p12.tile([128, ROWS], MDT, tag="q1T")
                nc.vector.memset(q0T[64:128, :], 0.0)
                nc.vector.memset(q1T[0:64, :], 0.0)
                qTs = (q0T, q1T)
                kT = p12.tile([DH, ROWS], MDT, tag="kT")
                # V_all[:, (tb*2 + h), 0:64] = V rows for global 128-token
                # block tb, head h; col 64 = ones (softmax denominator).
                V_all = p12.tile([128, 2 * KB * HPC, HD + 1], MDT, tag="V_all")
                nc.vector.memset(V_all[:, :, HD:HD + 1], 1.0)
                yT = p12.tile([DH, ROWS], MDT, tag="yT")

                # ---------------- phase 1: qkv projection + rope ----------
                with (
                    nc.named_scope("qkv"),
                    tc.tile_pool(name="ph1", bufs=1) as ph1,
                    tc.tile_pool(name="ph1ps", bufs=1, space="PSUM") as ph1ps,
                ):
                    wq_sb = [ph1.tile([128, DH], MDT, tag=f"wq{ci}", name=f"wq{ci}") for ci in range(NCI)]
                    wk_sb = [ph1.tile([128, DH], MDT, tag=f"wk{ci}", name=f"wk{ci}") for ci in range(NCI)]
                    wv_sb = [ph1.tile([128, DH], MDT, tag=f"wv{ci}", name=f"wv{ci}") for ci in range(NCI)]
                    C_sb = ph1.tile([DH, T], MDT, tag="ropeC")
                    S_sb = ph1.tile([DH, T], MDT, tag="ropeS")

                    PART = [1, 0, 3, 2]  # rope half-rotation partner groups
                    P1C = 512  # phase-1 psum chunk width (1 PSUM bank)
                    XW = 1024  # xq tile width
                    for QB in range(ROWS // XW):
                        bcols = slice(QB * XW, (QB + 1) * XW)
                        tcol = (QB * XW) % T  # column in the rope table
                        if QB * XW < T:
                            # rope table slices just-in-time (scalar ring)
                            nc.scalar.dma_start(
                                C_sb[:, QB * XW:(QB + 1) * XW],
                                ropeC_t[:, QB * XW:(QB + 1) * XW])
                            nc.scalar.dma_start(
                                S_sb[:, QB * XW:(QB + 1) * XW],
                                ropeS_t[:, QB * XW:(QB + 1) * XW])
                        xqs = []
                        for ci in range(NCI):
                            ring = RINGS[ci % 3]
                            if QB == 0:
                                # weight loads interleaved with first use
                                sl = slice(ci * 128, (ci + 1) * 128)
                                ring.dma_start(wq_sb[ci][:], wq_t[sl, :])
                                ring.dma_start(wk_sb[ci][:], wk_t[sl, :])
                                ring.dma_start(wv_sb[ci][:], wv_t[sl, :])
                            xq = ph1.tile([128, XW], MDT, tag=f"xq{ci}",
                                          bufs=2, name=f"xq{ci}")
                            ring.dma_start(
                                xq[:], xT_t[ci * 128:(ci + 1) * 128, bcols])
                            xqs.append(xq)
                            if QB == 0 and ci == 0:
                                nc.sync.dma_start(ones_row[:], ones_t[:])
                                if with_bias:
                                    nc.sync.dma_start(bqc[:], bqc_t[:])
                                    nc.sync.dma_start(bkc[:], bkc_t[:])
                                    nc.sync.dma_start(bvc[:], bvc_t[:])

                        for hf in range(XW // P1C):
                            hs = slice(hf * P1C, (hf + 1) * P1C)
                            cols = slice(QB * XW + hf * P1C,
                                         QB * XW + (hf + 1) * P1C)
                            tcols = slice(tcol + hf * P1C,
                                          tcol + (hf + 1) * P1C)
                            qps = ph1ps.tile([128, P1C], DT, tag="qps",
                                             bufs=2)
                            kps = ph1ps.tile([128, P1C], DT, tag="kps",
                                             bufs=2)
                            vps = ph1ps.tile([128, P1C], DT, tag="vps",
                                             bufs=2)
                            for ci in range(NCI):
                                st = ci == 0
                                sp = ci == NCI - 1
                                nc.tensor.matmul(qps[:], wq_sb[ci][:],
                                                 xqs[ci][:, hs],
                                                 start=st, stop=sp)
                                nc.tensor.matmul(kps[:], wk_sb[ci][:],
                                                 xqs[ci][:, hs],
                                                 start=st, stop=sp)
                                nc.tensor.matmul(vps[:], wv_sb[ci][:],
                                                 xqs[ci][:, hs],
                                                 start=st, stop=sp)

                            # rope: out = (q+b)*C + rot32(q+b)*S, fused with
                            # psum->sbuf eviction
                            for ps_tile, bias_n, dsts in (
                                    (qps, "bqc", None), (kps, "bkc", kT)):
                                ta = ph1.tile([128, P1C], MDT, tag="ta",
                                              bufs=2)
                                tb_ = ph1.tile([128, P1C], MDT, tag="tb",
                                               bufs=2)
                                if with_bias:
                                    bias_c = bqc if bias_n == "bqc" else bkc
                                    nc.vector.tensor_scalar(
                                        ps_tile[:], ps_tile[:], bias_c[:],
                                        None, mybir.AluOpType.add)
                                nc.vector.tensor_tensor(
                                    ta[:], ps_tile[:], C_sb[:, tcols],
                                    mybir.AluOpType.mult)
                                for g in range(4):
                                    gs = slice(32 * g, 32 * g + 32)
                                    prt = slice(32 * PART[g],
                                                32 * PART[g] + 32)
                                    nc.vector.tensor_tensor(
                                        tb_[gs, :], ps_tile[prt, :],
                                        S_sb[gs, tcols],
                                        mybir.AluOpType.mult)
                                if dsts is None:
                                    nc.gpsimd.tensor_tensor(
                                        q0T[0:64, cols], ta[0:64, :],
                                        tb_[0:64, :], mybir.AluOpType.add)
                                    nc.gpsimd.tensor_tensor(
                                        q1T[64:128, cols], ta[64:128, :],
                                        tb_[64:128, :], mybir.AluOpType.add)
                                else:
                                    nc.gpsimd.tensor_tensor(
                                        dsts[:, cols], ta[:], tb_[:],
                                        mybir.AluOpType.add)

                            # V: copy out (ACT) then transpose into V_all
                            vt = ph1.tile([128, P1C], MDT, tag="vt", bufs=2)
                            if with_bias:
                                nc.scalar.activation(
                                    vt[:], vps[:],
                                    mybir.ActivationFunctionType.Identity,
                                    bias=bvc[:])
                            else:
                                nc.scalar.copy(vt[:], vps[:])
                            for tb in range(P1C // 128):
                                gtb = (QB * XW + hf * P1C) // 128 + tb
                                for h in range(HPC):
                                    vap = ph1ps.tile([128, HD], MDT,
                                                     tag="vap", bufs=2)
                                    nc.tensor.transpose(
                                        vap[:],
                                        vt[h * HD:(h + 1) * HD,
                                           tb * 128:(tb + 1) * 128],
                                        ident[h * HD:(h + 1) * HD, :])
                                    nc.scalar.copy(
                                        V_all[:, gtb * HPC + h, 0:HD],
                                        vap[:])

                # load w_proj during attention (off the startup critical path)
                for ci in range(NCI):
                    nc.sync.dma_start(wp_sb[ci][:],
                                      wp_t[ci * 128:(ci + 1) * 128, :])
                nc.sync.dma_start(bp[:], bp_t[:])

                # ---------------- phase 2: causal attention ---------------
                # Software-pipelined: `delayed` holds closures (prev chunk's
                # A*V matmuls, per-qb finalizes, per-(b,h) A2A staging) that
                # are drained one per chunk so ACT/DVE hides under PE.
                with (
                    nc.named_scope("attn"),
                    tc.tile_pool(name="ph2", bufs=1) as ph2,
                    tc.tile_pool(name="ph2ps", bufs=1, space="PSUM") as ph2ps,
                ):
                    from collections import deque
                    delayed = deque()

                    def drain_all():
                        while delayed:
                            delayed.popleft()()

                    def make_finalize(oacc_u, h_u, bT_u, qb_u):
                        hp_u = slice(h_u * HD, (h_u + 1) * HD)

                        def fin():
                            linv = ph2.tile([1, 512], DT, tag="linv",
                                            bufs=3, name="linv")
                            # recip_approx (custom DVE) misreads PSUM inputs
                            # on HW: stage the denominator in SBUF first
                            dsb = ph2.tile([1, 512], DT, tag="dsb",
                                           bufs=3, name="dsb")
                            nc.vector.tensor_copy(
                                dsb[:], oacc_u[qb_u][HD:HD + 1, :])
                            nc.vector.reciprocal_approx_fast(linv[:], dsb[:])
                            rsb = ph2.tile([64, 512], DT, tag="rsb",
                                           bufs=3, name="rsb")
                            nc.gpsimd.partition_broadcast(
                                rsb[:], linv[:], channels=64)
                            nc.vector.tensor_tensor(
                                yT[hp_u, bT_u + qb_u * 512:
                                   bT_u + (qb_u + 1) * 512],
                                oacc_u[qb_u][0:HD, :], rsb[:],
                                mybir.AluOpType.mult)
                        return fin

                    def make_a2a(b_u, h_u, load_yr):
                        hp_u = slice(h_u * HD, (h_u + 1) * HD)

                        def stage():
                            for j in range(NCORES):
                                nc.sync.dma_start(
                                    a2a_in[b_u][h_u][j],
                                    yT[hp_u, b_u * T + j * RPB:
                                       b_u * T + (j + 1) * RPB])
                            if use_collective:
                                nc.gpsimd.collective_compute(
                                    "AllToAll", mybir.AluOpType.bypass,
                                    replica_groups=[list(range(NCORES))],
                                    ins=[a2a_in[b_u][h_u].opt()],
                                    outs=[a2a_out[b_u][h_u].opt()])
                            else:
                                nc.sync.dma_start(a2a_out[b_u][h_u][:],
                                                  a2a_in[b_u][h_u][:])
                            if load_yr:
                                for ci in range(NCORES):
                                    nc.sync.dma_start(
                                        yr[b_u][h_u * HD:(h_u + 1) * HD,
                                                ci, :],
                                        a2a_out[b_u][h_u][ci])
                        return stage

                    for bi, b in enumerate(BORDER):
                        for h in range(HPC):
                            hp = slice(h * HD, (h + 1) * HD)
                            bT = b * T
                            oacc = [ph2ps.tile([HD + 1, 512], DT,
                                               tag=f"oacc{qb}",
                                               name=f"oacc{qb}")
                                    for qb in range(T // 512)]
                            for kb in range(KB):
                                qs = kb * 128
                                # full-K stationary: both heads' kT rows; the
                                # moving q is zero in the other head's rows
                                lhs_k = kT[:, bT + qs:bT + qs + 128]
                                off = qs
                                while off < T:
                                    cw = min(QCH, T - off)
                                    qoff = off
                                    off += cw
                                    sps = ph2ps.tile([128, QCH], DT,
                                                     tag="strip", bufs=2,
                                                     name="sps")
                                    for po in range(0, cw, 512):
                                        w = min(512, cw - po)
                                        nc.tensor.matmul(
                                            sps[:, po:po + w], lhs_k,
                                            qTs[h][:, bT + qoff + po:
                                                   bT + qoff + po + w],
                                            start=True, stop=True)
                                    # bounded 2-deep pipeline: spread fin
                                    # bursts across a couple of chunks
                                    while len(delayed) > 2:
                                        delayed.popleft()()
                                    psb = ph2.tile([128, QCH], MDT, tag="psb",
                                                   bufs=4, name="psb")
                                    nc.scalar.activation(
                                        psb[:, 0:cw], sps[:, 0:cw],
                                        mybir.ActivationFunctionType.Exp,
                                        scale=1.0 / float(np.sqrt(HD)))
                                    if qoff == qs:
                                        # zero strict upper triangle (k > q)
                                        # on DVE: keeps the gpsimd queue free
                                        # for collectives (no head-of-line
                                        # stall on the next unit's masks)
                                        nc.vector.tensor_tensor(
                                            psb[:, 0:128], psb[:, 0:128],
                                            maskD[:], mybir.AluOpType.mult)

                                    def make_av(oacc_u=oacc, psb_u=psb,
                                                kb_u=kb, qoff_u=qoff,
                                                cw_u=cw, b_u=b, h_u=h):
                                        def av():
                                            vidx = ((b_u * KB + kb_u) * HPC
                                                    + h_u)
                                            for qb in range(T // 512):
                                                lo = max(qoff_u, qb * 512)
                                                hi = min(qoff_u + cw_u,
                                                         qb * 512 + 512)
                                                if lo >= hi:
                                                    continue
                                                nc.tensor.matmul(
                                                    oacc_u[qb][:,
                                                               lo - qb * 512:
                                                               hi - qb * 512],
                                                    V_all[:, vidx, :],
                                                    psb_u[:, lo - qoff_u:
                                                          hi - qoff_u],
                                                    start=(kb_u == 0),
                                                    stop=(kb_u == 4 * qb + 3))
                                        return av

                                    delayed.append(make_av())
                                # early finalize: oacc[qb] stops accumulating
                                # at kb == 4*qb+3 (its stop lands in this
                                # strip's first chunk)
                                if kb % 4 == 3:
                                    delayed.append(
                                        make_finalize(oacc, h, bT, kb // 4))
                            delayed.append(
                                make_a2a(b, h, load_yr=(bi == 0)))
                    drain_all()
                    if DEBUG_TAPS:
                        nc.sync.dma_start(qT_dbg[0:64, :], q0T[0:64, :])
                        nc.sync.dma_start(qT_dbg[64:128, :], q1T[64:128, :])
                        nc.sync.dma_start(kT_dbg[:], kT[:])
                        nc.sync.dma_start(yT_dbg[:], yT[:])

            # ---------------- phase 3: output projection ------------------
            with (
                nc.named_scope("proj"),
                tc.tile_pool(name="ph3", bufs=1) as ph3,
                tc.tile_pool(name="ph3ps", bufs=2, space="PSUM") as ph3ps,
            ):
                for bi, b in enumerate(BORDER):
                    if bi == 1:
                        for h in range(HPC):
                            for ci in range(NCORES):
                                nc.sync.dma_start(
                                    yr[b][h * HD:(h + 1) * HD, ci, :],
                                    a2a_out[b][h][ci])
                    for tb in range(RPB // 128):
                        for co in range(C // 512):
                            pps = ph3ps.tile([128, 512], DT, tag="pps",
                                             name="pps")
                            for ci in range(NCI):
                                nc.tensor.matmul(
                                    pps[:],
                                    yr[b][:, ci, tb * 128:(tb + 1) * 128],
                                    wp_sb[ci][:, co * 512:(co + 1) * 512],
                                    start=(ci == 0), stop=False)
                            nc.tensor.matmul(pps[:], ones_row[:, 0:128],
                                             bp[:, co * 512:(co + 1) * 512],
                                             start=False, stop=True)
                            osb = ph3.tile([128, 512], DT, tag="osb", bufs=2,
                                           name="osb")
                            nc.vector.tensor_copy(osb[:], pps[:])
                            nc.sync.dma_start(
                                out_t[b * RPB + tb * 128:
                                      b * RPB + (tb + 1) * 128,
                                      co * 512:(co + 1) * 512], osb[:])

    nc.compile()
    return nc


_NC_CACHE = {}


def _get_module(with_bias):
    if with_bias not in _NC_CACHE:
        _NC_CACHE[with_bias] = _build_module(with_bias)
    return _NC_CACHE[with_bias]


def _rope_tables():
    inv = ROPE_BASE ** (-np.arange(HALF, dtype=np.float64) / HALF)
    tt = np.arange(T, dtype=np.float64)
    ang = tt[None, :] * inv[:, None]  # [32, T]
    cos = np.cos(ang).astype(FP)  # [32, T]
    sin = np.sin(ang).astype(FP)
    Cq = np.concatenate([cos, cos], axis=0)  # [64, T] (p%32 freq)
    Sq = np.concatenate([-sin, sin], axis=0)
    # duplicate for the HPC heads (partition dim); columns span one batch
    Cq = np.tile(Cq, (HPC, 1))
    Sq = np.tile(Sq, (HPC, 1))
    return np.ascontiguousarray(Cq), np.ascontiguousarray(Sq)


def kernel(x, w_attn, b_attn, w_proj, b_proj, _trace=False):
    x = np.asarray(x, dtype=FP)
    w_attn = np.asarray(w_attn, dtype=FP)
    b_attn = np.asarray(b_attn, dtype=FP)
    w_proj = np.asarray(w_proj, dtype=FP)
    b_proj = np.asarray(b_proj, dtype=FP)

    with_bias = bool(np.any(b_attn))  # q/k/v biases (bp always applied)

    xT = np.ascontiguousarray(x.reshape(ROWS, C).T).astype(BF)  # [C, ROWS]
    ropeC, ropeS = _rope_tables()
    ropeC = ropeC.astype(BF)
    ropeS = ropeS.astype(BF)
    bp = np.ascontiguousarray(b_proj[None, :]).astype(BF)
    ones512 = np.ones((1, 512), BF)
    wp_bf = w_proj.astype(BF)

    in_maps = []
    for c in range(NCORES):
        h0 = HPC * c
        cols = slice(h0 * HD, (h0 + HPC) * HD)  # this core's head channels
        m = {
            "xT": xT,
            "wq": np.ascontiguousarray(w_attn[:, 0 * C:1 * C][:, cols]).astype(BF),
            "wk": np.ascontiguousarray(w_attn[:, 1 * C:2 * C][:, cols]).astype(BF),
            "wv": np.ascontiguousarray(w_attn[:, 2 * C:3 * C][:, cols]).astype(BF),
            "wp": wp_bf,
            "bp": bp,
            "ones512": ones512,
            "ropeC": ropeC,
            "ropeS": ropeS,
        }
        if with_bias:
            m["bqc"] = np.ascontiguousarray(b_attn[0 * C:1 * C][cols][:, None])
            m["bkc"] = np.ascontiguousarray(b_attn[1 * C:2 * C][cols][:, None])
            m["bvc"] = np.ascontiguousarray(b_attn[2 * C:3 * C][cols][:, None])
        in_maps.append(m)

    nc = _get_module(with_bias)
    res = run_bass_kernel_spmd(nc, in_maps, core_ids=list(range(NCORES)),
                               trace=_trace)
    # core c returns [2*RPB, C]: batch-0 rows [RPB*c, RPB*(c+1)), then the
    # same rows of batch 1
    out = np.empty((B, T, C), dtype=FP)
    for c in range(NCORES):
        o = res.results[c]["out"]
        for b in range(B):
            out[b, RPB * c:RPB * (c + 1), :] = o[b * RPB:(b + 1) * RPB]
    kernel.last_results = res
    return out
